# revision 21
# baseline (speedup 1.0000x reference)
"""Fused pre-norm transformer block on 8 Trainium2 NeuronCores.

Problem: x[4,1024,1024]; LN -> qkv attention (16 heads) -> proj + residual;
LN -> fc1 -> gelu -> fc2 + residual.  dense_transformer, compute regime.

Sharding (no collectives): 2 cores per batch element, each owning 512 rows.
Each core redundantly computes K/V for its whole batch (key order is
softmax-invariant), so attention, MLP and both residuals are fully
core-local.  The host passes each core its batch's rows with the core's own
512 rows first; outputs are reassembled on host.

Numerics: LayerNorm and the attention-score chain run in float32r
(TF32-class); V / attn / proj / MLP run in bf16 with fp32 PSUM
accumulation.  LN rsqrt = exp(-0.5*ln(var+eps)) + one Newton step; Ln/Exp
share one ACT table set with the softmax Exp, so the only table switch in
the whole kernel is Gelu before fc1.

Scheduling notes (HAM clock gate): the PE runs at 1.2 GHz until ~3.4us of
sustained matmul activity and re-throttles after ~3.4us idle.  PE-mode
transposes do NOT count as activity.  So: a warmup matmul burst at t=0,
Q-projection matmuls interleaved into the second half of LN1, and the exp
stream widened to [128,1024] PSUM reads to keep the attention pipeline
dense.  wf1 is prefetched on the scalar DMA queue during attention, wf2 on
the vector queue during proj, so the MLP never waits on HBM.
"""

import numpy as np
import ml_dtypes
from contextlib import ExitStack

import concourse.bass as bass
import concourse.tile as tile
from concourse import mybir
from concourse.bass_utils import run_bass_kernel_spmd
from concourse.vector_clock import ScopedClock, VectorClock
from concourse.masks import make_identity

F32 = mybir.dt.float32
F32R = mybir.dt.float32r
BF16 = mybir.dt.bfloat16
AF = mybir.ActivationFunctionType
OP = mybir.AluOpType
AX = mybir.AxisListType

B, N, C = 4, 1024, 1024
HEADS, DK = 16, 64
DFF = 4096
EPS = 1e-5
NB = 1024   # rows per core's batch (attention keys)
NO = 512    # rows owned per core
P = 128
CCH = C // P      # 8 chunks over C
MCH = NB // P     # 8 key-row chunks
OCH = NO // P     # 4 own-row chunks
FFCH = DFF // P   # 32
NHALF = NB // 512


class SplitDrainTileContext(tile.TileContext):
    """This walrus build rejects >2 sync waits on the tail SP drain
    ("Too many sync wait commands"); split the global-clock waits across
    single-wait drain instructions."""

    def _drain_and_barrier(self, tick_clock, wait_clock):
        nc = self.nc
        gc = tick_clock.global_clock
        n = len(gc)
        for i in range(n):
            if gc[i] > 0:
                vc = VectorClock([0] * n)
                vc.require_at_least(i, gc[i])
                d = nc.sync.drain()
                wait_clock.add_sem_waits(d.ins, ScopedClock({None: vc}))
        nc.sync.drain()
        nc.all_engine_barrier()
        popped = nc._tile_sem_poison_stack.pop()
        assert popped is self._sem_poison
        nc.clear_and_free_semaphores(list(self.sems.allocated().values()))
        nc.all_engine_barrier()


def legalize_waits(nc, cap=1):
    """Walrus here allows at most one sync wait per regular instruction.
    Hoist excess waits onto same-engine NoOps placed just before."""
    n = [0]

    def mknop(engine, wait):
        n[0] += 1
        nop = mybir.InstNoOp(name=f"I-waitfix-{n[0]}", ins=[], outs=[])
        nop.engine = engine
        nop.sync_info = mybir.SyncInfo(on_wait=[wait], on_update=[])
        return nop

    for f in nc.m.functions:
        for bb in f.blocks:
            out = []
            for inst in bb.instructions:
                w = list(inst.sync_info.on_wait or []) if inst.sync_info else []
                if len(w) > cap:
                    for extra in w[:-cap]:
                        out.append(mknop(inst.engine, extra))
                    inst.sync_info.on_wait = w[-cap:]
                out.append(inst)
            bb.instructions = out


def _rsqrt(nc, pool, var, eps, newton=1):
    """[128,1] fp32: 1/sqrt(var+eps) = exp(-0.5*ln(var+eps)) + Newton.
    Ln/Exp share a table set with the softmax Exp -> no table switch."""
    a = pool.tile([P, 1], F32, tag="rsq_a")
    nc.vector.tensor_scalar_add(a[:], var, eps)
    l0 = pool.tile([P, 1], F32, tag="rsq_l")
    nc.scalar.activation(l0[:], a[:], AF.Ln)
    r = pool.tile([P, 1], F32, tag="rsq_r")
    nc.scalar.activation(r[:], l0[:], AF.Exp, scale=-0.5)
    t = pool.tile([P, 1], F32, tag="rsq_t")
    for _ in range(newton):
        nc.vector.tensor_mul(t[:], r[:], r[:])
        nc.vector.tensor_mul(t[:], t[:], a[:])
        nc.vector.tensor_scalar(t[:], t[:], -0.5, 1.5, op0=OP.mult, op1=OP.add)
        nc.vector.tensor_mul(r[:], r[:], t[:])
    return r


def _layer_norm_chunk(nc, pool, x_i, xn_i, newton=1):
    """LN of one [128, C] row-chunk: xn_i = (x - mean(x)) * rsqrt(var+eps)."""
    nsub = C // 512
    stats = pool.tile([P, nsub, 6], F32, tag="ln_stats")
    for s in range(nsub):
        nc.vector.bn_stats(stats[:, s, :], x_i[:, s * 512:(s + 1) * 512])
    mv = pool.tile([P, 2], F32, tag="ln_mv")
    nc.vector.bn_aggr(mv[:], stats[:])
    r = _rsqrt(nc, pool, mv[:, 1:2], EPS, newton=newton)
    nc.vector.tensor_scalar(
        xn_i[:], x_i[:], mv[:, 0:1], r[:], op0=OP.subtract, op1=OP.mult
    )


def build_program(legalize=True):
    nc = bass.Bass()

    x = nc.declare_dram_parameter("x", [NB, C], F32, isOutput=False)
    # weight tensors arrive host-pretransposed so every DMA slab is
    # contiguous per partition: w_qk[slab*128+p2, c*128+q], etc.
    w_qk = nc.declare_dram_parameter("w_qk", [16 * P, C], F32R, isOutput=False)
    w_v = nc.declare_dram_parameter("w_v", [2 * P, CCH * 512], F32R, isOutput=False)
    w_proj = nc.declare_dram_parameter("w_proj", [P, CCH * C], BF16, isOutput=False)
    w_fc1 = nc.declare_dram_parameter("w_fc1", [FFCH * P, C], BF16, isOutput=False)
    w_fc2 = nc.declare_dram_parameter("w_fc2", [DFF, C], BF16, isOutput=False)
    b_qk = nc.declare_dram_parameter("b_qk", [2 * C], F32, isOutput=False)
    b_v = nc.declare_dram_parameter("b_v", [C], F32, isOutput=False)
    b_proj = nc.declare_dram_parameter("b_proj", [C], F32, isOutput=False)
    b_fc1 = nc.declare_dram_parameter("b_fc1", [DFF], F32, isOutput=False)
    b_fc2 = nc.declare_dram_parameter("b_fc2", [C], F32, isOutput=False)
    out = nc.declare_dram_parameter("out", [NO, C], F32, isOutput=True)

    with SplitDrainTileContext(nc) as tc:
        with ExitStack() as ctx:
            _build_body(
                nc, tc, ctx,
                x, w_qk, w_v, w_proj, w_fc1, w_fc2,
                b_qk, b_v, b_proj, b_fc1, b_fc2, out,
            )
    if legalize:
        legalize_waits(nc)
    return nc


def _build_body(nc, tc, ctx, x, w_qk, w_v, w_proj, w_fc1, w_fc2,
                b_qk, b_v, b_proj, b_fc1, b_fc2, out):
    perm = ctx.enter_context(tc.tile_pool(name="perm", bufs=1))
    small = ctx.enter_context(tc.tile_pool(name="small", bufs=3))

    # --- constants / biases ------------------------------------------------
    ident = perm.tile([P, P], F32)
    make_identity(nc, ident[:])

    bqk_sb = perm.tile([P, 2 * CCH], F32)
    nc.gpsimd.dma_start(bqk_sb[:], b_qk.rearrange("(c p) -> p c", p=P))
    bfc1_sb = perm.tile([P, FFCH], F32)
    nc.gpsimd.dma_start(bfc1_sb[:], b_fc1.rearrange("(c p) -> p c", p=P))
    bfc2_bc = perm.tile([P, C], F32)
    nc.gpsimd.dma_start(
        bfc2_bc[:], bass.AP(tensor=b_fc2[:].tensor, offset=b_fc2[:].offset,
                            ap=[[0, P], [1, C]]))

    # ones row in DRAM scratch (for the kt65 shift-fold row), staged at t=0
    gdrp = ctx.enter_context(tc.tile_pool(name="gdrp", bufs=1, space="DRAM"))
    ones_dr = gdrp.tile([1, NB], F32)
    ones_sb = small.tile([1, NB], F32, tag="ones_sb")
    nc.vector.memset(ones_sb[:], 1.0)
    nc.sync.dma_start(ones_dr[:], ones_sb[:])

    # --- PE warmup: dense matmuls release the HAM clock gate before the
    # first real matmul stream (transposes do not count as PE activity).
    wrm = perm.tile([P, 512], BF16)
    nc.vector.memset(wrm[:], 0.25)
    with ExitStack() as wu:
        wups = wu.enter_context(tc.tile_pool(name="wups", bufs=1, space="PSUM"))
        wt = wups.tile([P, 512], F32)
        for _ in range(30):
            nc.tensor.matmul(wt[:], wrm[:, 0:P], wrm[:], start=True, stop=True)

    x2 = perm.tile([P, OCH, C], BF16)         # post-attn residual stream

    with ExitStack() as kqv_scope:
        kqv = kqv_scope.enter_context(tc.tile_pool(name="kqv", bufs=1))
        x_own = kqv.tile([P, OCH, C], BF16)     # own rows (residual 1)
        kT = kqv.tile([P, CCH, NB], F32R)       # K^T head-pairs [128=2*dk, m]
        qT = kqv.tile([P, CCH, NO], F32R)       # Q^T head-pairs [128=2*dk, n_own]
        v_sb = kqv.tile([P, MCH, HEADS, DK + 1], BF16)  # V rows + ones col
        ctxT = kqv.tile([P, CCH, NO], BF16)     # (attn@V)^T, normalized

        def _max_unit(half, pp_, ncc, qk1pool, stg_h):
            base = ((pp_ - half * (CCH // 2)) * 2) * OCH + ncc
            ps1a = qk1pool.tile([P, NB], F32, tag="ps1")
            ps1b = qk1pool.tile([P, NB], F32, tag="ps1")
            for mh in range(NHALF):
                nc.tensor.matmul(
                    ps1a[:, mh * 512:(mh + 1) * 512],
                    qT[0:DK, pp_, ncc * P:(ncc + 1) * P],
                    kT[0:DK, pp_, mh * 512:(mh + 1) * 512],
                    start=True, stop=True)
                nc.tensor.matmul(
                    ps1b[:, mh * 512:(mh + 1) * 512],
                    qT[DK:P, pp_, ncc * P:(ncc + 1) * P],
                    kT[DK:P, pp_, mh * 512:(mh + 1) * 512],
                    start=True, stop=True)
            nc.vector.reduce_max(
                stg_h[:, base:base + 1], ps1a[:], axis=AX.X, negate=True)
            nc.vector.reduce_max(
                stg_h[:, base + OCH:base + OCH + 1], ps1b[:],
                axis=AX.X, negate=True)

        def _max_finish(half, stg_h, trps):
            pstg = trps.tile([CCH * OCH, P], F32, tag="pstg", name=f"pstg{half}")
            nc.tensor.transpose(pstg[:], stg_h[:], ident[:])
            nc.vector.tensor_copy(stage2[half][:], pstg[0:CCH * OCH, :])
        stage2 = [kqv.tile([CCH * OCH, P], F32R, name=f"stage2_{i}")
                  for i in range(2)]

        with ExitStack() as ph_a:
            # ========== S1/S2: x, LN1, transpose; Q; K; max both halves; V ==
            xnt_pool = ph_a.enter_context(tc.tile_pool(name="xnt", bufs=1))
            xnT = xnt_pool.tile([P, CCH, NB], F32R)   # LN1(x)^T  [c, n]
            bv_bc = xnt_pool.tile([P, C], F32)
            nc.gpsimd.dma_start(
                bv_bc[:], bass.AP(tensor=b_v[:].tensor, offset=b_v[:].offset,
                                  ap=[[0, P], [1, C]]))
            ln = ph_a.enter_context(tc.tile_pool(name="ln", bufs=4))
            xoth = ph_a.enter_context(tc.tile_pool(name="xoth", bufs=2))
            tps = ph_a.enter_context(tc.tile_pool(name="tps", bufs=1, space="PSUM"))
            qkps = ph_a.enter_context(tc.tile_pool(name="qkps", bufs=2, space="PSUM"))
            pstgps = ph_a.enter_context(tc.tile_pool(name="pstgps", bufs=1, space="PSUM"))
            qk1ps = ph_a.enter_context(tc.tile_pool(name="qk1ps", bufs=2, space="PSUM"))
            wq = ph_a.enter_context(tc.tile_pool(name="wq", bufs=3))
            sm1 = ph_a.enter_context(tc.tile_pool(name="sm1", bufs=4))

            def _ln_chunk(i):
                x_i = xoth.tile([P, C], F32, tag="x_i")
                (nc.sync if i % 2 == 0 else nc.scalar).dma_start(
                    x_i[:], x[i * P:(i + 1) * P, :])
                if i < OCH:
                    nc.scalar.copy(x_own[:, i, :], x_i[:])
                xn_i = xoth.tile([P, C], F32, tag="xn_i")
                _layer_norm_chunk(nc, ln, x_i, xn_i)
                for c in range(CCH):
                    pst = tps.tile([P, P], F32)
                    nc.tensor.transpose(pst[:], xn_i[:, c * P:(c + 1) * P], ident[:])
                    nc.vector.tensor_copy(xnT[:, c, i * P:(i + 1) * P], pst[:])
                # keep real-MM activity flowing for the HAM activity monitor
                for _ in range(2):
                    wps_ = qkps.tile([P, 512], F32, tag="qk_ps")
                    nc.tensor.matmul(wps_[:], wrm[:, 0:P], wrm[:],
                                     start=True, stop=True)

            # own rows first: Q only needs these
            for i in range(OCH):
                _ln_chunk(i)

            # Q^T per pair p (rhs = xnT rows 0:512, ready now).  One slab DMA
            # per p; this matmul stream keeps HAM warm during LN1's key-half.
            for p in range(CCH):
                wq_t = wq.tile([P, CCH, P], F32R, tag="wqk_t")
                nc.sync.dma_start(
                    wq_t[:],
                    w_qk[p * P:(p + 1) * P, :].rearrange("p2 (c q) -> p2 c q", q=P))
                ps = qkps.tile([P, 512], F32, tag="qk_ps")
                for c in range(CCH):
                    nc.tensor.matmul(ps[:], wq_t[:, c, :], xnT[:, c, 0:NO],
                                     start=(c == 0), stop=(c == CCH - 1))
                nc.scalar.activation(qT[:, p, :], ps[:], AF.Identity,
                                     bias=bqk_sb[:, p:p + 1])
                if p < OCH:
                    _ln_chunk(OCH + p)

            # K^T per pair p over all 1024 keys
            for p in range(CCH):
                wk_t = wq.tile([P, CCH, P], F32R, tag="wqk_t")
                nc.sync.dma_start(
                    wk_t[:],
                    w_qk[(CCH + p) * P:(CCH + p + 1) * P, :].rearrange(
                        "p2 (c q) -> p2 c q", q=P))
                for nh in range(NHALF):
                    ps = qkps.tile([P, 512], F32, tag="qk_ps")
                    for c in range(CCH):
                        nc.tensor.matmul(
                            ps[:], wk_t[:, c, :], xnT[:, c, nh * 512:(nh + 1) * 512],
                            start=(c == 0), stop=(c == CCH - 1))
                    nc.scalar.activation(
                        kT[:, p, nh * 512:(nh + 1) * 512], ps[:], AF.Identity,
                        bias=bqk_sb[:, CCH + p:CCH + p + 1])

            # full max pass (both halves) interleaved with V: the V matmul
            # stream fills the PE while DVE drains the row-max reduces, so
            # HAM stays warm and attention starts immediately after.
            wv = ph_a.enter_context(tc.tile_pool(name="wv", bufs=2))
            for mc in range(MCH):
                nc.vector.memset(v_sb[:, mc, :, DK:DK + 1], 1.0)
            stg_hs = [sm1.tile([P, CCH * OCH], F32, tag=f"stg{i}",
                               name=f"stg{i}") for i in range(2)]
            wvhs = []
            for dh in range(2):
                wvh = wv.tile([P, CCH, 512], F32R, tag="wvh", name=f"wvh{dh}")
                nc.scalar.dma_start(
                    wvh[:],
                    w_v[dh * P:(dh + 1) * P, :].rearrange(
                        "p2 (c q) -> p2 c q", q=512))
                wvhs.append(wvh)

            def _v_chunk(k):
                dh, mc = k // MCH, k % MCH
                ps = qkps.tile([P, 512], F32, tag="qk_ps")
                for c in range(CCH):
                    nc.tensor.matmul(
                        ps[:], xnT[:, c, mc * P:(mc + 1) * P], wvhs[dh][:, c, :],
                        start=(c == 0), stop=(c == CCH - 1))
                nc.vector.tensor_tensor(
                    out=v_sb[:, mc, dh * 8:(dh + 1) * 8, 0:DK],
                    in0=ps[:].rearrange("p (h d) -> p h d", d=DK),
                    in1=bv_bc[:, dh * 512:(dh + 1) * 512].rearrange(
                        "p (h d) -> p h d", d=DK),
                    op=OP.add,
                )

            vk = 0
            for half in range(2):
                for u, (pp_, ncc) in enumerate(
                        (p_, n_) for p_ in range(half * 4, half * 4 + 4)
                        for n_ in range(OCH)):
                    _max_unit(half, pp_, ncc, qk1ps, stg_hs[half])
                    if u % 2 == 1:
                        _v_chunk(vk)
                        vk += 1
                _max_finish(half, stg_hs[half], pstgps)

        # ================ S3: attention + proj ============================
        # Software-pipelined head loop: head h's scores/exp interleave with
        # head h-1's PV matmuls at 2-chunk granularity, so the PE never
        # idles while ScalarE runs exp.  proj accumulates row-form into SBUF
        # per completed head pair (no transposes).  kt65/qt65 assembly rides
        # the gpsimd DMA queue; the softmax reciprocal broadcast bounces
        # through DRAM scratch on the sync queue.
        wpb_scope = ExitStack()
        wpb = wpb_scope.enter_context(tc.tile_pool(name="wpb", bufs=1))
        wpfull = wpb.tile([P, CCH, C], BF16)
        bproj_bc = wpb.tile([P, C], F32)
        p_acc = wpb.tile([P, OCH, C], F32)

        with ExitStack() as ph_b:
            kq65 = ph_b.enter_context(tc.tile_pool(name="kq65", bufs=3))
            att = ph_b.enter_context(tc.tile_pool(name="att", bufs=4))
            sm = ph_b.enter_context(tc.tile_pool(name="sm", bufs=3))
            rbc = ph_b.enter_context(tc.tile_pool(name="rbc", bufs=3))
            drp = ph_b.enter_context(tc.tile_pool(name="drp", bufs=3, space="DRAM"))
            qk2ps = ph_b.enter_context(
                tc.tile_pool(name="qk2ps", bufs=2, space="PSUM"))
            ctxps = ph_b.enter_context(
                tc.tile_pool(name="ctxps", bufs=3, space="PSUM"))

            info = {}
            pend = {}

            def _assemble(h):
                half, hh = h // CCH, h % CCH
                pp, off = h // 2, (h % 2) * DK
                kt65 = kq65.tile([DK + 1, NB], F32R, tag="kt65")
                qt65 = kq65.tile([DK + 1, NO], F32R, tag="qt65")
                nc.gpsimd.dma_start(kt65[0:DK, :], kT[off:off + DK, pp, :])
                nc.gpsimd.dma_start(kt65[DK:DK + 1, :], ones_dr[:])
                nc.gpsimd.dma_start(qt65[0:DK, :], qT[off:off + DK, pp, :])
                nc.sync.dma_start(qt65[DK:DK + 1, :],
                                  stage2[half][hh * OCH:(hh + 1) * OCH, :])
                return kt65, qt65

            def _denom(h):
                ps3 = info[h]["ps3"]
                s_row = sm.tile([1, 512], F32, tag="s_row")
                nc.vector.tensor_copy(s_row[:], ps3[DK:DK + 1, :])
                s_scr = drp.tile([1, 512], F32, tag="s_scr")
                nc.gpsimd.dma_start(s_scr[:], s_row[:])
                s_sq = sm.tile([P, 4], F32, tag="s_sq")
                nc.gpsimd.dma_start(
                    s_sq[:],
                    bass.AP(tensor=s_scr.tensor, offset=s_scr.offset,
                            ap=[[4, P], [1, 4]]))
                r_sq = sm.tile([P, 4], F32, tag="r_sq")
                nc.vector.reciprocal(r_sq[:], s_sq[:])
                r_scr = drp.tile([1, 512], F32, tag="r_scr")
                nc.sync.dma_start(
                    bass.AP(tensor=r_scr.tensor, offset=r_scr.offset,
                            ap=[[4, P], [1, 4]]), r_sq[:])
                r_bc = rbc.tile([DK, 512], F32, tag="r_bc")
                nc.sync.dma_start(
                    r_bc[:],
                    bass.AP(tensor=r_scr.tensor, offset=r_scr.offset,
                            ap=[[0, DK], [1, 512]]))
                pend[h] = (ps3, r_bc)

            def _emit_norm(h):
                off2 = (h % 2) * DK
                ps3h, r_bch = pend.pop(h)
                nc.vector.tensor_tensor(
                    out=ctxT[off2:off2 + DK, h // 2, :], in0=ps3h[0:DK, :],
                    in1=r_bch[:], op=OP.mult)

            def _pv_step(h, j):
                g = info[h]
                for jj in range(2):
                    mc = 2 * j + jj
                    nc.tensor.matmul(
                        g["ps3"][:], v_sb[:, mc, h, :], g["at"][:, mc, :],
                        start=(mc == 0), stop=(mc == MCH - 1))

            for h in range(HEADS):
                kt65, qt65 = _assemble(h)
                attnT = att.tile([P, MCH, 512], BF16, tag="attnT")
                info[h] = {"at": attnT}
                if h >= 1:
                    ps3_ = ctxps.tile([DK + 1, 512], F32, tag="ps3",
                                      name=f"ps3_{h - 1}")
                    info[h - 1]["ps3"] = ps3_
                for j in range(MCH // 2):
                    ps2 = qk2ps.tile([P, 2, 512], F32, tag="ps2")
                    for jj in range(2):
                        mc = 2 * j + jj
                        nc.tensor.matmul(
                            ps2[:, jj, :], kt65[:, mc * P:(mc + 1) * P], qt65[:],
                            start=True, stop=True)
                    if h >= 1:
                        _pv_step(h - 1, j)
                    nc.scalar.activation(
                        attnT[:, 2 * j:2 * j + 2, :], ps2[:], AF.Exp)
                if h >= 1:
                    _denom(h - 1)
                if h >= 2:
                    _emit_norm(h - 2)
                if h == 2:
                    # proj weights + residual base, loaded mid-attention so
                    # the early heads' kt65/qt65 DMAs go first
                    nc.gpsimd.dma_start(
                        wpfull[:], w_proj.rearrange("p2 (c q) -> p2 c q", q=C))
                    nc.gpsimd.dma_start(
                        bproj_bc[:],
                        bass.AP(tensor=b_proj[:].tensor, offset=b_proj[:].offset,
                                ap=[[0, P], [1, C]]))
                    for r in range(OCH):
                        nc.vector.tensor_tensor(
                            out=p_acc[:, r, :], in0=x_own[:, r, :],
                            in1=bproj_bc[:], op=OP.add)
            # drain: PV + denom of head 15, final norms, last proj pair
            ps3_last = ctxps.tile([DK + 1, 512], F32, tag="ps3", name="ps3_last")
            info[HEADS - 1]["ps3"] = ps3_last
            for j in range(MCH // 2):
                _pv_step(HEADS - 1, j)
            _denom(HEADS - 1)
            _emit_norm(HEADS - 2)
            _emit_norm(HEADS - 1)

        # proj: row-form, PSUM-accumulated over head pairs (no transposes)
        with ExitStack() as ph_c:
            prps = ph_c.enter_context(tc.tile_pool(name="prps", bufs=1, space="PSUM"))
            pracc = [prps.tile([P, 2, 512], F32, tag=f"pr{r}", name=f"pr{r}")
                     for r in range(OCH)]
            for pp in range(CCH):
                for r in range(OCH):
                    for hf in range(2):
                        nc.tensor.matmul(
                            pracc[r][:, hf, :], ctxT[:, pp, r * P:(r + 1) * P],
                            wpfull[:, pp, hf * 512:(hf + 1) * 512],
                            start=(pp == 0), stop=(pp == CCH - 1))
            for r in range(OCH):
                nc.vector.tensor_tensor(
                    out=x2[:, r, :], in0=p_acc[:, r, :],
                    in1=pracc[r][:].rearrange("p a b -> p (a b)"), op=OP.add)
        wpb_scope.close()

    # ================ S4b: LN2 + transpose; MLP ===========================
    with ExitStack() as mlp_scope:
        mlp = mlp_scope.enter_context(tc.tile_pool(name="mlp", bufs=1))
        x2nT = mlp.tile([P, CCH, NO], BF16)
        hT = mlp.tile([P, FFCH, NO], BF16)
        x3_pre = mlp.tile([P, OCH, C], F32)       # x2 + b_fc2 (residual base)
        with ExitStack() as ph_d:
            ln2 = ph_d.enter_context(tc.tile_pool(name="ln2", bufs=2))
            tps3 = ph_d.enter_context(tc.tile_pool(name="tps3", bufs=4, space="PSUM"))
            for i in range(OCH):
                x2n_i = ln2.tile([P, C], F32, tag="x2n_i")
                _layer_norm_chunk(nc, ln2, x2[:, i, :], x2n_i, newton=1)
                for c in range(CCH):
                    pst = tps3.tile([P, P], F32)
                    nc.tensor.transpose(
                        pst[:], x2n_i[:, c * P:(c + 1) * P], ident[:])
                    nc.vector.tensor_copy(x2nT[:, c, i * P:(i + 1) * P], pst[:])
                nc.vector.tensor_tensor(
                    out=x3_pre[:, i, :], in0=x2[:, i, :], in1=bfc2_bc[:],
                    op=OP.add)

        # ================ S5: fc1 + gelu ==================================
        with ExitStack() as ph_e:
            f1ps = ph_e.enter_context(tc.tile_pool(name="f1ps", bufs=4, space="PSUM"))
            wf1s = ph_e.enter_context(tc.tile_pool(name="wf1s", bufs=6))
            for ff in range(FFCH):
                wsl = wf1s.tile([P, CCH, P], BF16, tag="wsl")
                nc.sync.dma_start(
                    wsl[:],
                    w_fc1[ff * P:(ff + 1) * P, :].rearrange(
                        "p2 (c q) -> p2 c q", q=P))
                ps = f1ps.tile([P, 512], F32)
                for c in range(CCH):
                    nc.tensor.matmul(
                        ps[:], wsl[:, c, :], x2nT[:, c, :],
                        start=(c == 0), stop=(c == CCH - 1))
                nc.scalar.activation(hT[:, ff, :], ps[:], AF.Gelu,
                                     bias=bfc1_sb[:, ff:ff + 1])
        # ================ S6: fc2 row-form + residual, store ==============
        with ExitStack() as ph_f:
            f2ps = ph_f.enter_context(tc.tile_pool(name="f2ps", bufs=1, space="PSUM"))
            sc2 = ph_f.enter_context(tc.tile_pool(name="sc2", bufs=3))
            wf2s = ph_f.enter_context(tc.tile_pool(name="wf2s", bufs=6))
            psacc = [f2ps.tile([P, C], F32, tag=f"f2acc{r}", name=f"f2acc{r}")
                     for r in range(OCH)]
            for ff in range(FFCH):
                w2 = wf2s.tile([P, C], BF16, tag="w2sl")
                nc.gpsimd.dma_start(w2[:], w_fc2[ff * P:(ff + 1) * P, :])
                for r in range(OCH):
                    for hf in range(2):
                        nc.tensor.matmul(
                            psacc[r][:, hf * 512:(hf + 1) * 512],
                            hT[:, ff, r * P:(r + 1) * P],
                            w2[:, hf * 512:(hf + 1) * 512],
                            start=(ff == 0), stop=(ff == FFCH - 1))
            for r in range(OCH):
                fin = sc2.tile([P, C], F32, tag="fin")
                nc.vector.tensor_tensor(
                    out=fin[:], in0=x3_pre[:, r, :], in1=psacc[r][:], op=OP.add)
                (nc.sync if r % 2 == 0 else nc.scalar).dma_start(
                    out[r * P:(r + 1) * P, :], fin[:])


_NC_CACHE = [None]


def _get_nc():
    if _NC_CACHE[0] is None:
        _NC_CACHE[0] = build_program()
    return _NC_CACHE[0]


def _prepare_in_maps(inputs):
    f32 = lambda a: np.ascontiguousarray(np.asarray(a, dtype=np.float32))
    x = f32(inputs["x"])
    g = f32(inputs["norm_g"])
    bb = f32(inputs["norm_b"])
    w_qkv = f32(inputs["w_qkv"])
    b_qkv = f32(inputs["b_qkv"])
    w_proj = f32(inputs["w_proj"])
    b_proj = f32(inputs["b_proj"])
    w_fc1 = f32(inputs["w_fc1"])
    b_fc1 = f32(inputs["b_fc1"])
    w_fc2 = f32(inputs["w_fc2"])
    b_fc2 = f32(inputs["b_fc2"])

    # fold the LN affine into the consuming matmuls; fold the sqrt(dk)
    # score scale into w_q/b_q
    w_qkv_f = w_qkv * g[:, None]
    b_qkv_f = b_qkv + bb @ w_qkv
    scale = float(DK) ** 0.5
    w_q = w_qkv_f[:, 0:C] * scale
    b_q = b_qkv_f[0:C] * scale
    w_k = w_qkv_f[:, C:2 * C]
    b_k = b_qkv_f[C:2 * C]
    w_v = np.ascontiguousarray(w_qkv_f[:, 2 * C:3 * C])
    b_v = np.ascontiguousarray(b_qkv_f[2 * C:3 * C])
    w_fc1_f = w_fc1 * g[:, None]
    b_fc1_f = b_fc1 + bb @ w_fc1

    bf = lambda a: np.ascontiguousarray(a.astype(ml_dtypes.bfloat16))
    # slab-major relayout: [slab, p2, c, q] so each kernel DMA reads
    # contiguous per-partition lines
    wqk_n = np.concatenate([w_q, w_k], axis=1)
    wqk_s = wqk_n.reshape(8, 128, 16, 128).transpose(2, 1, 0, 3).reshape(2048, 1024)
    wv_s = w_v.reshape(8, 128, 2, 512).transpose(2, 1, 0, 3).reshape(256, 4096)
    wp_s = w_proj.reshape(8, 128, 1024).transpose(1, 0, 2).reshape(128, 8192)
    wf1_s = w_fc1_f.reshape(8, 128, 32, 128).transpose(2, 1, 0, 3).reshape(4096, 1024)
    shared = {
        "w_qk": np.ascontiguousarray(wqk_s),
        "w_v": np.ascontiguousarray(wv_s),
        "w_proj": bf(wp_s),
        "w_fc1": bf(wf1_s),
        "w_fc2": bf(w_fc2),
        "b_qk": np.ascontiguousarray(np.concatenate([b_q, b_k])),
        "b_v": b_v,
        "b_proj": b_proj,
        "b_fc1": np.ascontiguousarray(b_fc1_f),
        "b_fc2": b_fc2,
    }
    in_maps = []
    for core in range(8):
        b, half = core // 2, core % 2
        xb = x[b]
        x_core = np.ascontiguousarray(np.concatenate(
            [xb[half * NO:(half + 1) * NO], xb[(1 - half) * NO:(2 - half) * NO]],
            axis=0))
        in_maps.append({"x": x_core, **shared})
    return in_maps


def kernel(**inputs) -> np.ndarray:
    nc = _get_nc()
    in_maps = _prepare_in_maps(inputs)
    res = run_bass_kernel_spmd(nc, in_maps, list(range(8)))
    out = np.empty((B, N, C), dtype=np.float32)
    for core in range(8):
        b, half = core // 2, core % 2
        out[b, half * NO:(half + 1) * NO] = res.results[core]["out"]
    return out


# revision 22
# speedup vs baseline: 1.0308x; 1.0308x over previous
"""Fused pre-norm transformer block on 8 Trainium2 NeuronCores.

Problem: x[4,1024,1024]; LN -> qkv attention (16 heads) -> proj + residual;
LN -> fc1 -> gelu -> fc2 + residual.  dense_transformer, compute regime.

Sharding (no collectives): 2 cores per batch element, each owning 512 rows.
Each core redundantly computes K/V for its whole batch (key order is
softmax-invariant), so attention, MLP and both residuals are fully
core-local.  The host passes each core its batch's rows with the core's own
512 rows first; outputs are reassembled on host.

Numerics: LayerNorm and the attention-score chain run in float32r
(TF32-class); V / attn / proj / MLP run in bf16 with fp32 PSUM
accumulation.  LN rsqrt = exp(-0.5*ln(var+eps)) + one Newton step; Ln/Exp
share one ACT table set with the softmax Exp, so the only table switch in
the whole kernel is Gelu before fc1.

Scheduling notes (HAM clock gate): the PE runs at 1.2 GHz until ~3.4us of
sustained matmul activity and re-throttles after ~3.4us idle.  PE-mode
transposes do NOT count as activity.  So: a warmup matmul burst at t=0,
Q-projection matmuls interleaved into the second half of LN1, and the exp
stream widened to [128,1024] PSUM reads to keep the attention pipeline
dense.  wf1 is prefetched on the scalar DMA queue during attention, wf2 on
the vector queue during proj, so the MLP never waits on HBM.
"""

import numpy as np
import ml_dtypes
from contextlib import ExitStack

import concourse.bass as bass
import concourse.tile as tile
from concourse import mybir
from concourse.bass_utils import run_bass_kernel_spmd
from concourse.vector_clock import ScopedClock, VectorClock
from concourse.masks import make_identity

F32 = mybir.dt.float32
F32R = mybir.dt.float32r
BF16 = mybir.dt.bfloat16
AF = mybir.ActivationFunctionType
OP = mybir.AluOpType
AX = mybir.AxisListType

B, N, C = 4, 1024, 1024
HEADS, DK = 16, 64
DFF = 4096
EPS = 1e-5
NB = 1024   # rows per core's batch (attention keys)
NO = 512    # rows owned per core
P = 128
CCH = C // P      # 8 chunks over C
MCH = NB // P     # 8 key-row chunks
OCH = NO // P     # 4 own-row chunks
FFCH = DFF // P   # 32
NHALF = NB // 512


class SplitDrainTileContext(tile.TileContext):
    """This walrus build rejects >2 sync waits on the tail SP drain
    ("Too many sync wait commands"); split the global-clock waits across
    single-wait drain instructions."""

    def _drain_and_barrier(self, tick_clock, wait_clock):
        nc = self.nc
        gc = tick_clock.global_clock
        n = len(gc)
        for i in range(n):
            if gc[i] > 0:
                vc = VectorClock([0] * n)
                vc.require_at_least(i, gc[i])
                d = nc.sync.drain()
                wait_clock.add_sem_waits(d.ins, ScopedClock({None: vc}))
        nc.sync.drain()
        nc.all_engine_barrier()
        popped = nc._tile_sem_poison_stack.pop()
        assert popped is self._sem_poison
        nc.clear_and_free_semaphores(list(self.sems.allocated().values()))
        nc.all_engine_barrier()


def legalize_waits(nc, cap=1):
    """Walrus here allows at most one sync wait per regular instruction.
    Hoist excess waits onto same-engine NoOps placed just before."""
    n = [0]

    def mknop(engine, wait):
        n[0] += 1
        nop = mybir.InstNoOp(name=f"I-waitfix-{n[0]}", ins=[], outs=[])
        nop.engine = engine
        nop.sync_info = mybir.SyncInfo(on_wait=[wait], on_update=[])
        return nop

    for f in nc.m.functions:
        for bb in f.blocks:
            out = []
            for inst in bb.instructions:
                w = list(inst.sync_info.on_wait or []) if inst.sync_info else []
                if len(w) > cap:
                    for extra in w[:-cap]:
                        out.append(mknop(inst.engine, extra))
                    inst.sync_info.on_wait = w[-cap:]
                out.append(inst)
            bb.instructions = out


def _rsqrt(nc, pool, var, eps, newton=1):
    """[128,1] fp32: 1/sqrt(var+eps) = exp(-0.5*ln(var+eps)) + Newton.
    Ln/Exp share a table set with the softmax Exp -> no table switch."""
    a = pool.tile([P, 1], F32, tag="rsq_a")
    nc.vector.tensor_scalar_add(a[:], var, eps)
    l0 = pool.tile([P, 1], F32, tag="rsq_l")
    nc.scalar.activation(l0[:], a[:], AF.Ln)
    r = pool.tile([P, 1], F32, tag="rsq_r")
    nc.scalar.activation(r[:], l0[:], AF.Exp, scale=-0.5)
    t = pool.tile([P, 1], F32, tag="rsq_t")
    for _ in range(newton):
        nc.vector.tensor_mul(t[:], r[:], r[:])
        nc.vector.tensor_mul(t[:], t[:], a[:])
        nc.vector.tensor_scalar(t[:], t[:], -0.5, 1.5, op0=OP.mult, op1=OP.add)
        nc.vector.tensor_mul(r[:], r[:], t[:])
    return r


def _layer_norm_chunk(nc, pool, x_i, xn_i, newton=1):
    """LN of one [128, C] row-chunk: xn_i = (x - mean(x)) * rsqrt(var+eps)."""
    nsub = C // 512
    stats = pool.tile([P, nsub, 6], F32, tag="ln_stats")
    for s in range(nsub):
        nc.vector.bn_stats(stats[:, s, :], x_i[:, s * 512:(s + 1) * 512])
    mv = pool.tile([P, 2], F32, tag="ln_mv")
    nc.vector.bn_aggr(mv[:], stats[:])
    r = _rsqrt(nc, pool, mv[:, 1:2], EPS, newton=newton)
    nc.vector.tensor_scalar(
        xn_i[:], x_i[:], mv[:, 0:1], r[:], op0=OP.subtract, op1=OP.mult
    )


def build_program(legalize=True):
    nc = bass.Bass()

    x = nc.declare_dram_parameter("x", [NB, C], F32, isOutput=False)
    # weight tensors arrive host-pretransposed so every DMA slab is
    # contiguous per partition: w_qk[slab*128+p2, c*128+q], etc.
    w_qk = nc.declare_dram_parameter("w_qk", [16 * P, C], F32R, isOutput=False)
    w_v = nc.declare_dram_parameter("w_v", [2 * P, CCH * 512], F32R, isOutput=False)
    w_proj = nc.declare_dram_parameter("w_proj", [P, CCH * C], BF16, isOutput=False)
    w_fc1 = nc.declare_dram_parameter("w_fc1", [FFCH * P, C], BF16, isOutput=False)
    w_fc2 = nc.declare_dram_parameter("w_fc2", [DFF, C], BF16, isOutput=False)
    b_qk = nc.declare_dram_parameter("b_qk", [2 * C], F32, isOutput=False)
    b_v = nc.declare_dram_parameter("b_v", [C], F32, isOutput=False)
    b_proj = nc.declare_dram_parameter("b_proj", [C], F32, isOutput=False)
    b_fc1 = nc.declare_dram_parameter("b_fc1", [DFF], F32, isOutput=False)
    b_fc2 = nc.declare_dram_parameter("b_fc2", [C], F32, isOutput=False)
    out = nc.declare_dram_parameter("out", [NO, C], F32, isOutput=True)

    with SplitDrainTileContext(nc) as tc:
        with ExitStack() as ctx:
            _build_body(
                nc, tc, ctx,
                x, w_qk, w_v, w_proj, w_fc1, w_fc2,
                b_qk, b_v, b_proj, b_fc1, b_fc2, out,
            )
    if legalize:
        legalize_waits(nc)
    return nc


def _build_body(nc, tc, ctx, x, w_qk, w_v, w_proj, w_fc1, w_fc2,
                b_qk, b_v, b_proj, b_fc1, b_fc2, out):
    perm = ctx.enter_context(tc.tile_pool(name="perm", bufs=1))
    small = ctx.enter_context(tc.tile_pool(name="small", bufs=3))

    # --- constants / biases ------------------------------------------------
    ident = perm.tile([P, P], F32)
    make_identity(nc, ident[:])

    bqk_sb = perm.tile([P, 2 * CCH], F32)
    nc.gpsimd.dma_start(bqk_sb[:], b_qk.rearrange("(c p) -> p c", p=P))
    bfc1_sb = perm.tile([P, FFCH], F32)
    nc.gpsimd.dma_start(bfc1_sb[:], b_fc1.rearrange("(c p) -> p c", p=P))
    bfc2_bc = perm.tile([P, C], F32)
    nc.gpsimd.dma_start(
        bfc2_bc[:], bass.AP(tensor=b_fc2[:].tensor, offset=b_fc2[:].offset,
                            ap=[[0, P], [1, C]]))

    # ones row in DRAM scratch (for the kt65 shift-fold row), staged at t=0
    gdrp = ctx.enter_context(tc.tile_pool(name="gdrp", bufs=1, space="DRAM"))
    ones_dr = gdrp.tile([1, NB], F32)
    ones_sb = small.tile([1, NB], F32, tag="ones_sb")
    nc.vector.memset(ones_sb[:], 1.0)
    nc.sync.dma_start(ones_dr[:], ones_sb[:])

    # --- PE warmup: dense matmuls release the HAM clock gate before the
    # first real matmul stream (transposes do not count as PE activity).
    wrm = perm.tile([P, 512], BF16)
    nc.vector.memset(wrm[:], 0.25)
    with ExitStack() as wu:
        wups = wu.enter_context(tc.tile_pool(name="wups", bufs=1, space="PSUM"))
        wt = wups.tile([P, 512], F32)
        for _ in range(30):
            nc.tensor.matmul(wt[:], wrm[:, 0:P], wrm[:], start=True, stop=True)

    x2 = perm.tile([P, OCH, C], BF16)         # post-attn residual stream

    with ExitStack() as kqv_scope:
        kqv = kqv_scope.enter_context(tc.tile_pool(name="kqv", bufs=1))
        x_own = kqv.tile([P, OCH, C], BF16)     # own rows (residual 1)
        kT = kqv.tile([P, CCH, NB], F32R)       # K^T head-pairs [128=2*dk, m]
        qT = kqv.tile([P, CCH, NO], F32R)       # Q^T head-pairs [128=2*dk, n_own]
        v_sb = kqv.tile([P, MCH, HEADS, DK + 1], BF16)  # V rows + ones col
        ctxT = kqv.tile([P, CCH, NO], BF16)     # (attn@V)^T, normalized

        def _max_unit(half, pp_, ncc, qk1pool, stg_h):
            base = ((pp_ - half * (CCH // 2)) * 2) * OCH + ncc
            ps1a = qk1pool.tile([P, NB], F32, tag="ps1")
            ps1b = qk1pool.tile([P, NB], F32, tag="ps1")
            for mh in range(NHALF):
                nc.tensor.matmul(
                    ps1a[:, mh * 512:(mh + 1) * 512],
                    qT[0:DK, pp_, ncc * P:(ncc + 1) * P],
                    kT[0:DK, pp_, mh * 512:(mh + 1) * 512],
                    start=True, stop=True)
                nc.tensor.matmul(
                    ps1b[:, mh * 512:(mh + 1) * 512],
                    qT[DK:P, pp_, ncc * P:(ncc + 1) * P],
                    kT[DK:P, pp_, mh * 512:(mh + 1) * 512],
                    start=True, stop=True)
            nc.vector.reduce_max(
                stg_h[:, base:base + 1], ps1a[:], axis=AX.X, negate=True)
            nc.vector.reduce_max(
                stg_h[:, base + OCH:base + OCH + 1], ps1b[:],
                axis=AX.X, negate=True)

        def _max_finish(half, stg_h, trps):
            pstg = trps.tile([CCH * OCH, P], F32, tag="pstg", name=f"pstg{half}")
            nc.tensor.transpose(pstg[:], stg_h[:], ident[:])
            nc.vector.tensor_copy(stage2[half][:], pstg[0:CCH * OCH, :])
        stage2 = [kqv.tile([CCH * OCH, P], F32R, name=f"stage2_{i}")
                  for i in range(2)]

        with ExitStack() as ph_a:
            # ========== S1/S2: x, LN1, transpose; Q; K; max both halves; V ==
            xnt_pool = ph_a.enter_context(tc.tile_pool(name="xnt", bufs=1))
            xnT = xnt_pool.tile([P, CCH, NB], F32R)   # LN1(x)^T  [c, n]
            bv_bc = xnt_pool.tile([P, C], F32)
            nc.gpsimd.dma_start(
                bv_bc[:], bass.AP(tensor=b_v[:].tensor, offset=b_v[:].offset,
                                  ap=[[0, P], [1, C]]))
            ln = ph_a.enter_context(tc.tile_pool(name="ln", bufs=4))
            xoth = ph_a.enter_context(tc.tile_pool(name="xoth", bufs=2))
            tps = ph_a.enter_context(tc.tile_pool(name="tps", bufs=1, space="PSUM"))
            qkps = ph_a.enter_context(tc.tile_pool(name="qkps", bufs=2, space="PSUM"))
            pstgps = ph_a.enter_context(tc.tile_pool(name="pstgps", bufs=1, space="PSUM"))
            qk1ps = ph_a.enter_context(tc.tile_pool(name="qk1ps", bufs=2, space="PSUM"))
            wq = ph_a.enter_context(tc.tile_pool(name="wq", bufs=3))
            sm1 = ph_a.enter_context(tc.tile_pool(name="sm1", bufs=4))

            def _ln_chunk(i):
                x_i = xoth.tile([P, C], F32, tag="x_i")
                (nc.sync if i % 2 == 0 else nc.scalar).dma_start(
                    x_i[:], x[i * P:(i + 1) * P, :])
                if i < OCH:
                    nc.scalar.copy(x_own[:, i, :], x_i[:])
                xn_i = xoth.tile([P, C], F32, tag="xn_i")
                _layer_norm_chunk(nc, ln, x_i, xn_i)
                for c in range(CCH):
                    pst = tps.tile([P, P], F32)
                    nc.tensor.transpose(pst[:], xn_i[:, c * P:(c + 1) * P], ident[:])
                    nc.vector.tensor_copy(xnT[:, c, i * P:(i + 1) * P], pst[:])
                # keep real-MM activity flowing for the HAM activity monitor
                for _ in range(2):
                    wps_ = qkps.tile([P, 512], F32, tag="qk_ps")
                    nc.tensor.matmul(wps_[:], wrm[:, 0:P], wrm[:],
                                     start=True, stop=True)

            # own rows first: Q only needs these
            for i in range(OCH):
                _ln_chunk(i)

            # Q^T per pair p (rhs = xnT rows 0:512, ready now).  One slab DMA
            # per p; this matmul stream keeps HAM warm during LN1's key-half.
            for p in range(CCH):
                wq_t = wq.tile([P, CCH, P], F32R, tag="wqk_t")
                nc.sync.dma_start(
                    wq_t[:],
                    w_qk[p * P:(p + 1) * P, :].rearrange("p2 (c q) -> p2 c q", q=P))
                ps = qkps.tile([P, 512], F32, tag="qk_ps")
                for c in range(CCH):
                    nc.tensor.matmul(ps[:], wq_t[:, c, :], xnT[:, c, 0:NO],
                                     start=(c == 0), stop=(c == CCH - 1))
                nc.scalar.activation(qT[:, p, :], ps[:], AF.Identity,
                                     bias=bqk_sb[:, p:p + 1])
                if p < OCH:
                    _ln_chunk(OCH + p)

            # K^T per pair p over all 1024 keys
            for p in range(CCH):
                wk_t = wq.tile([P, CCH, P], F32R, tag="wqk_t")
                nc.sync.dma_start(
                    wk_t[:],
                    w_qk[(CCH + p) * P:(CCH + p + 1) * P, :].rearrange(
                        "p2 (c q) -> p2 c q", q=P))
                for nh in range(NHALF):
                    ps = qkps.tile([P, 512], F32, tag="qk_ps")
                    for c in range(CCH):
                        nc.tensor.matmul(
                            ps[:], wk_t[:, c, :], xnT[:, c, nh * 512:(nh + 1) * 512],
                            start=(c == 0), stop=(c == CCH - 1))
                    nc.scalar.activation(
                        kT[:, p, nh * 512:(nh + 1) * 512], ps[:], AF.Identity,
                        bias=bqk_sb[:, CCH + p:CCH + p + 1])

            # full max pass (both halves) interleaved with V: the V matmul
            # stream fills the PE while DVE drains the row-max reduces, so
            # HAM stays warm and attention starts immediately after.
            wv = ph_a.enter_context(tc.tile_pool(name="wv", bufs=2))
            for mc in range(MCH):
                nc.vector.memset(v_sb[:, mc, :, DK:DK + 1], 1.0)
            stg_hs = [sm1.tile([P, CCH * OCH], F32, tag=f"stg{i}",
                               name=f"stg{i}") for i in range(2)]
            wvhs = []
            for dh in range(2):
                wvh = wv.tile([P, CCH, 512], F32R, tag="wvh", name=f"wvh{dh}")
                nc.scalar.dma_start(
                    wvh[:],
                    w_v[dh * P:(dh + 1) * P, :].rearrange(
                        "p2 (c q) -> p2 c q", q=512))
                wvhs.append(wvh)

            def _v_chunk(k):
                dh, mc = k // MCH, k % MCH
                ps = qkps.tile([P, 512], F32, tag="qk_ps")
                for c in range(CCH):
                    nc.tensor.matmul(
                        ps[:], xnT[:, c, mc * P:(mc + 1) * P], wvhs[dh][:, c, :],
                        start=(c == 0), stop=(c == CCH - 1))
                nc.vector.tensor_tensor(
                    out=v_sb[:, mc, dh * 8:(dh + 1) * 8, 0:DK],
                    in0=ps[:].rearrange("p (h d) -> p h d", d=DK),
                    in1=bv_bc[:, dh * 512:(dh + 1) * 512].rearrange(
                        "p (h d) -> p h d", d=DK),
                    op=OP.add,
                )

            vk = 0
            for half in range(2):
                for u, (pp_, ncc) in enumerate(
                        (p_, n_) for p_ in range(half * 4, half * 4 + 4)
                        for n_ in range(OCH)):
                    _max_unit(half, pp_, ncc, qk1ps, stg_hs[half])
                    if u % 2 == 1:
                        _v_chunk(vk)
                        vk += 1
                _max_finish(half, stg_hs[half], pstgps)

        # ================ S3: attention + proj ============================
        # Software-pipelined head loop: head h's scores/exp interleave with
        # head h-1's PV matmuls at 2-chunk granularity, so the PE never
        # idles while ScalarE runs exp.  proj accumulates row-form into SBUF
        # per completed head pair (no transposes).  kt65/qt65 assembly rides
        # the gpsimd DMA queue; the softmax reciprocal broadcast bounces
        # through DRAM scratch on the sync queue.
        wpb_scope = ExitStack()
        wpb = wpb_scope.enter_context(tc.tile_pool(name="wpb", bufs=1))
        wpfull = wpb.tile([P, CCH, C], BF16)
        bproj_bc = wpb.tile([P, C], F32)
        p_acc = wpb.tile([P, OCH, C], F32)

        with ExitStack() as ph_b:
            kq65 = ph_b.enter_context(tc.tile_pool(name="kq65", bufs=3))
            att = ph_b.enter_context(tc.tile_pool(name="att", bufs=4))
            sm = ph_b.enter_context(tc.tile_pool(name="sm", bufs=3))
            rbc = ph_b.enter_context(tc.tile_pool(name="rbc", bufs=3))
            drp = ph_b.enter_context(tc.tile_pool(name="drp", bufs=3, space="DRAM"))
            qk2ps = ph_b.enter_context(
                tc.tile_pool(name="qk2ps", bufs=2, space="PSUM"))
            ctxps = ph_b.enter_context(
                tc.tile_pool(name="ctxps", bufs=3, space="PSUM"))

            info = {}
            pend = {}

            def _assemble(h):
                half, hh = h // CCH, h % CCH
                pp, off = h // 2, (h % 2) * DK
                kt65 = kq65.tile([DK + 1, NB], F32R, tag="kt65")
                qt65 = kq65.tile([DK + 1, NO], F32R, tag="qt65")
                nc.gpsimd.dma_start(kt65[0:DK, :], kT[off:off + DK, pp, :])
                nc.gpsimd.dma_start(kt65[DK:DK + 1, :], ones_dr[:])
                nc.gpsimd.dma_start(qt65[0:DK, :], qT[off:off + DK, pp, :])
                nc.sync.dma_start(qt65[DK:DK + 1, :],
                                  stage2[half][hh * OCH:(hh + 1) * OCH, :])
                return kt65, qt65

            def _denom(h):
                ps3 = info[h]["ps3"]
                s_row = sm.tile([1, 512], F32, tag="s_row")
                nc.vector.tensor_copy(s_row[:], ps3[DK:DK + 1, :])
                s_scr = drp.tile([1, 512], F32, tag="s_scr")
                nc.sync.dma_start(s_scr[:], s_row[:])
                s_sq = sm.tile([P, 4], F32, tag="s_sq")
                nc.sync.dma_start(
                    s_sq[:],
                    bass.AP(tensor=s_scr.tensor, offset=s_scr.offset,
                            ap=[[4, P], [1, 4]]))
                r_sq = sm.tile([P, 4], F32, tag="r_sq")
                nc.vector.reciprocal(r_sq[:], s_sq[:])
                r_scr = drp.tile([1, 512], F32, tag="r_scr")
                nc.sync.dma_start(
                    bass.AP(tensor=r_scr.tensor, offset=r_scr.offset,
                            ap=[[4, P], [1, 4]]), r_sq[:])
                r_bc = rbc.tile([DK, 512], F32, tag="r_bc")
                nc.sync.dma_start(
                    r_bc[:],
                    bass.AP(tensor=r_scr.tensor, offset=r_scr.offset,
                            ap=[[0, DK], [1, 512]]))
                pend[h] = (ps3, r_bc)

            def _emit_norm(h):
                off2 = (h % 2) * DK
                ps3h, r_bch = pend.pop(h)
                nc.vector.tensor_tensor(
                    out=ctxT[off2:off2 + DK, h // 2, :], in0=ps3h[0:DK, :],
                    in1=r_bch[:], op=OP.mult)

            def _pv_step(h, j):
                g = info[h]
                for jj in range(2):
                    mc = 2 * j + jj
                    nc.tensor.matmul(
                        g["ps3"][:], v_sb[:, mc, h, :], g["at"][:, mc, :],
                        start=(mc == 0), stop=(mc == MCH - 1))

            for h in range(HEADS):
                kt65, qt65 = _assemble(h)
                attnT = att.tile([P, MCH, 512], BF16, tag="attnT")
                info[h] = {"at": attnT}
                if h >= 1:
                    ps3_ = ctxps.tile([DK + 1, 512], F32, tag="ps3",
                                      name=f"ps3_{h - 1}")
                    info[h - 1]["ps3"] = ps3_
                for j in range(MCH // 2):
                    ps2 = qk2ps.tile([P, 2, 512], F32, tag="ps2")
                    for jj in range(2):
                        mc = 2 * j + jj
                        nc.tensor.matmul(
                            ps2[:, jj, :], kt65[:, mc * P:(mc + 1) * P], qt65[:],
                            start=True, stop=True)
                    if h >= 1:
                        _pv_step(h - 1, j)
                    nc.scalar.activation(
                        attnT[:, 2 * j:2 * j + 2, :], ps2[:], AF.Exp)
                if h >= 1:
                    _denom(h - 1)
                if h >= 2:
                    _emit_norm(h - 2)
                if h == 2:
                    # proj weights + residual base, loaded mid-attention so
                    # the early heads' kt65/qt65 DMAs go first
                    nc.gpsimd.dma_start(
                        wpfull[:], w_proj.rearrange("p2 (c q) -> p2 c q", q=C))
                    nc.gpsimd.dma_start(
                        bproj_bc[:],
                        bass.AP(tensor=b_proj[:].tensor, offset=b_proj[:].offset,
                                ap=[[0, P], [1, C]]))
                    for r in range(OCH):
                        nc.vector.tensor_tensor(
                            out=p_acc[:, r, :], in0=x_own[:, r, :],
                            in1=bproj_bc[:], op=OP.add)
            # drain: PV + denom of head 15, final norms, last proj pair
            ps3_last = ctxps.tile([DK + 1, 512], F32, tag="ps3", name="ps3_last")
            info[HEADS - 1]["ps3"] = ps3_last
            for j in range(MCH // 2):
                _pv_step(HEADS - 1, j)
            _denom(HEADS - 1)
            _emit_norm(HEADS - 2)
            _emit_norm(HEADS - 1)

        # proj: row-form, PSUM-accumulated over head pairs (no transposes)
        with ExitStack() as ph_c:
            prps = ph_c.enter_context(tc.tile_pool(name="prps", bufs=1, space="PSUM"))
            pracc = [prps.tile([P, 2, 512], F32, tag=f"pr{r}", name=f"pr{r}")
                     for r in range(OCH)]
            for pp in range(CCH):
                for r in range(OCH):
                    for hf in range(2):
                        nc.tensor.matmul(
                            pracc[r][:, hf, :], ctxT[:, pp, r * P:(r + 1) * P],
                            wpfull[:, pp, hf * 512:(hf + 1) * 512],
                            start=(pp == 0), stop=(pp == CCH - 1))
            for r in range(OCH):
                nc.vector.tensor_tensor(
                    out=x2[:, r, :], in0=p_acc[:, r, :],
                    in1=pracc[r][:].rearrange("p a b -> p (a b)"), op=OP.add)
        wpb_scope.close()

    # ================ S4b: LN2 + transpose; MLP ===========================
    with ExitStack() as mlp_scope:
        mlp = mlp_scope.enter_context(tc.tile_pool(name="mlp", bufs=1))
        x2nT = mlp.tile([P, CCH, NO], BF16)
        hT = mlp.tile([P, FFCH, NO], BF16)
        x3_pre = mlp.tile([P, OCH, C], F32)       # x2 + b_fc2 (residual base)
        with ExitStack() as ph_d:
            ln2 = ph_d.enter_context(tc.tile_pool(name="ln2", bufs=2))
            tps3 = ph_d.enter_context(tc.tile_pool(name="tps3", bufs=4, space="PSUM"))
            for i in range(OCH):
                x2n_i = ln2.tile([P, C], F32, tag="x2n_i")
                _layer_norm_chunk(nc, ln2, x2[:, i, :], x2n_i, newton=1)
                for c in range(CCH):
                    pst = tps3.tile([P, P], F32)
                    nc.tensor.transpose(
                        pst[:], x2n_i[:, c * P:(c + 1) * P], ident[:])
                    nc.vector.tensor_copy(x2nT[:, c, i * P:(i + 1) * P], pst[:])
                nc.vector.tensor_tensor(
                    out=x3_pre[:, i, :], in0=x2[:, i, :], in1=bfc2_bc[:],
                    op=OP.add)

        # ================ S5: fc1 + gelu ==================================
        with ExitStack() as ph_e:
            f1ps = ph_e.enter_context(tc.tile_pool(name="f1ps", bufs=4, space="PSUM"))
            wf1s = ph_e.enter_context(tc.tile_pool(name="wf1s", bufs=6))
            for ff in range(FFCH):
                wsl = wf1s.tile([P, CCH, P], BF16, tag="wsl")
                nc.sync.dma_start(
                    wsl[:],
                    w_fc1[ff * P:(ff + 1) * P, :].rearrange(
                        "p2 (c q) -> p2 c q", q=P))
                ps = f1ps.tile([P, 512], F32)
                for c in range(CCH):
                    nc.tensor.matmul(
                        ps[:], wsl[:, c, :], x2nT[:, c, :],
                        start=(c == 0), stop=(c == CCH - 1))
                nc.scalar.activation(hT[:, ff, :], ps[:], AF.Gelu,
                                     bias=bfc1_sb[:, ff:ff + 1])
        # ================ S6: fc2 row-form + residual, store ==============
        with ExitStack() as ph_f:
            f2ps = ph_f.enter_context(tc.tile_pool(name="f2ps", bufs=1, space="PSUM"))
            sc2 = ph_f.enter_context(tc.tile_pool(name="sc2", bufs=3))
            wf2s = ph_f.enter_context(tc.tile_pool(name="wf2s", bufs=6))
            psacc = [f2ps.tile([P, C], F32, tag=f"f2acc{r}", name=f"f2acc{r}")
                     for r in range(OCH)]
            for ff in range(FFCH):
                w2 = wf2s.tile([P, C], BF16, tag="w2sl")
                nc.gpsimd.dma_start(w2[:], w_fc2[ff * P:(ff + 1) * P, :])
                for r in range(OCH):
                    for hf in range(2):
                        nc.tensor.matmul(
                            psacc[r][:, hf * 512:(hf + 1) * 512],
                            hT[:, ff, r * P:(r + 1) * P],
                            w2[:, hf * 512:(hf + 1) * 512],
                            start=(ff == 0), stop=(ff == FFCH - 1))
            for r in range(OCH):
                fin = sc2.tile([P, C], F32, tag="fin")
                nc.vector.tensor_tensor(
                    out=fin[:], in0=x3_pre[:, r, :], in1=psacc[r][:], op=OP.add)
                (nc.sync if r % 2 == 0 else nc.scalar).dma_start(
                    out[r * P:(r + 1) * P, :], fin[:])


_NC_CACHE = [None]


def _get_nc():
    if _NC_CACHE[0] is None:
        _NC_CACHE[0] = build_program()
    return _NC_CACHE[0]


def _prepare_in_maps(inputs):
    f32 = lambda a: np.ascontiguousarray(np.asarray(a, dtype=np.float32))
    x = f32(inputs["x"])
    g = f32(inputs["norm_g"])
    bb = f32(inputs["norm_b"])
    w_qkv = f32(inputs["w_qkv"])
    b_qkv = f32(inputs["b_qkv"])
    w_proj = f32(inputs["w_proj"])
    b_proj = f32(inputs["b_proj"])
    w_fc1 = f32(inputs["w_fc1"])
    b_fc1 = f32(inputs["b_fc1"])
    w_fc2 = f32(inputs["w_fc2"])
    b_fc2 = f32(inputs["b_fc2"])

    # fold the LN affine into the consuming matmuls; fold the sqrt(dk)
    # score scale into w_q/b_q
    w_qkv_f = w_qkv * g[:, None]
    b_qkv_f = b_qkv + bb @ w_qkv
    scale = float(DK) ** 0.5
    w_q = w_qkv_f[:, 0:C] * scale
    b_q = b_qkv_f[0:C] * scale
    w_k = w_qkv_f[:, C:2 * C]
    b_k = b_qkv_f[C:2 * C]
    w_v = np.ascontiguousarray(w_qkv_f[:, 2 * C:3 * C])
    b_v = np.ascontiguousarray(b_qkv_f[2 * C:3 * C])
    w_fc1_f = w_fc1 * g[:, None]
    b_fc1_f = b_fc1 + bb @ w_fc1

    bf = lambda a: np.ascontiguousarray(a.astype(ml_dtypes.bfloat16))
    # slab-major relayout: [slab, p2, c, q] so each kernel DMA reads
    # contiguous per-partition lines
    wqk_n = np.concatenate([w_q, w_k], axis=1)
    wqk_s = wqk_n.reshape(8, 128, 16, 128).transpose(2, 1, 0, 3).reshape(2048, 1024)
    wv_s = w_v.reshape(8, 128, 2, 512).transpose(2, 1, 0, 3).reshape(256, 4096)
    wp_s = w_proj.reshape(8, 128, 1024).transpose(1, 0, 2).reshape(128, 8192)
    wf1_s = w_fc1_f.reshape(8, 128, 32, 128).transpose(2, 1, 0, 3).reshape(4096, 1024)
    shared = {
        "w_qk": np.ascontiguousarray(wqk_s),
        "w_v": np.ascontiguousarray(wv_s),
        "w_proj": bf(wp_s),
        "w_fc1": bf(wf1_s),
        "w_fc2": bf(w_fc2),
        "b_qk": np.ascontiguousarray(np.concatenate([b_q, b_k])),
        "b_v": b_v,
        "b_proj": b_proj,
        "b_fc1": np.ascontiguousarray(b_fc1_f),
        "b_fc2": b_fc2,
    }
    in_maps = []
    for core in range(8):
        b, half = core // 2, core % 2
        xb = x[b]
        x_core = np.ascontiguousarray(np.concatenate(
            [xb[half * NO:(half + 1) * NO], xb[(1 - half) * NO:(2 - half) * NO]],
            axis=0))
        in_maps.append({"x": x_core, **shared})
    return in_maps


def kernel(**inputs) -> np.ndarray:
    nc = _get_nc()
    in_maps = _prepare_in_maps(inputs)
    res = run_bass_kernel_spmd(nc, in_maps, list(range(8)))
    out = np.empty((B, N, C), dtype=np.float32)
    for core in range(8):
        b, half = core // 2, core % 2
        out[b, half * NO:(half + 1) * NO] = res.results[core]["out"]
    return out
